# revision 8
# baseline (speedup 1.0000x reference)
"""Trainium2 Bass kernel for nn_BinaryMLP (BitNet-ternary SwiGLU MLP).

reference math (fp32):
    s_i = mean(|w_i|)            (per-tensor scalar, i in {1,3,2})
    wq_i = clip(round(w_i/s_i), -1, 1) * s_i     (ternary * scale)
    h1 = x @ w1q.T ; h3 = x @ w3q.T
    y  = (silu(h1) * h3) @ w2q.T

Strategy (8 cores, data-parallel over the 16384 tokens).  Per core:
  phase A: |w| partial sums over this core's h-column shard of w1/w3 and
           d-row shard of w2 (DVE abs-reduce, GPSIMD partition reduce) ->
           tiny 8-core AllReduce -> per-partition ternarize thresholds.
  tern:    w1/w3 are sharded across cores by h-TILES (core j owns tiles
           own_start[j]..+own_n[j]-1).  Slot-0 tiles of every core are
           additionally replicated in fp32 to every core, so each core
           ternarizes tiles {own_start[j]} locally and phase B starts
           with 8 tiles of runway before the single AllGather (slots
           1..5 of every core) has to land.  Ternary values are exact
           {-1,0,+1} in fp8; per-tensor scales fold into the epilogue.
  phase B: h-tile loop over the 43 tiles in a fixed processed order
           (slot-0 tiles first), each tile = 128 h rows x 2048 tokens:
           z1/z3 matmuls vs resident bf16 x (fp8 ternary weights
           stationary, 8 psum banks), epilogue split across engines so
           nothing head-of-line blocks: silu on Act, z3 scaling on Act
           (Copy+scale), the g product on GPSIMD, ternarize only on DVE.
           g goes to DRAM in processed-tile order, split into two
           tensors so phase C's first token-tile loads start mid-B.
           w2 is ternarized+AllGathered and half of it prefetched to
           SBUF during B (baseline scheme).
  phase C: y[m,d] = sum_h g[h,m] w2q[h,d], g stationary, fp32 PSUM, d in
           two halves (half A SBUF-resident early, half B streamed).
  A long calibrated warm-up matmul stream keeps the PE HAM-warm through
  phase A so real matmuls start at full clock.

All arithmetic (scales, ternarization, matmuls) happens on device; the
host only reshapes / transposes / pads / slices / concatenates.
"""

import sys
from contextlib import ExitStack

import numpy as np

if "/opt/trn_rl_repo" not in sys.path:
    sys.path.insert(0, "/opt/trn_rl_repo")

import concourse.bass as bass  # noqa: E402,F401
import concourse.bass_isa as bass_isa  # noqa: E402
import concourse.mybir as mybir  # noqa: E402
import concourse.tile as tile  # noqa: E402
from concourse import bacc  # noqa: E402

F32 = mybir.dt.float32
BF16 = mybir.dt.bfloat16
FP8 = mybir.dt.float8e4
AF = mybir.ActivationFunctionType
ALU = mybir.AluOpType
AX = mybir.AxisListType

# Full problem geometry (hardcoded per contest rules).
B, S, D = 4, 4096, 2048
H_REAL = 5461
HP = 5504            # H padded to 43*128
N_CORES = 8
M = (B * S) // N_CORES   # tokens per core = 2048
HT = HP // 128           # 43 h-tiles
KD = D // 128            # 16 k-tiles over D
MC = M // 512            # 4 moving chunks of 512 tokens

# h-tile ownership (global, same table baked into every core's program)
OWN_N = [5, 5, 5, 5, 5, 6, 6, 6]
OWN_START = [0, 5, 10, 15, 20, 25, 31, 37]
assert sum(OWN_N) == HT
SLOTS = 6                 # padded slots per core (zero-pad for own_n=5)

# processed-tile order: slot-0 tiles of all cores first (locally
# ternarized from the replicated input -> zero collective latency),
# then slots 1..5 per core (delivered by the one AllGather).
PROC = [OWN_START[j] for j in range(N_CORES)] + [
    OWN_START[j] + s
    for j in range(N_CORES)
    for s in range(1, OWN_N[j])
]
assert sorted(PROC) == list(range(HT))
N_GA = 22                 # processed tiles 0..21 -> g_a, rest -> g_b
N_WARM = 320              # warm-up matmuls (keeps PE busy through phase A)


def build_module(d=D, m=M, n_cores=N_CORES, h_real=H_REAL):
    kd, ht, mc = KD, HT, MC
    n_true = h_real * d
    sw = d * HP // (n_cores * 128)   # w2 slice free elems per partition
    shw = SLOTS * 128                # 768 padded own h-cols of w1/w3

    nc = bacc.Bacc(
        "TRN2",
        target_bir_lowering=False,
        debug=False,
        num_devices=n_cores,
    )
    xT = nc.dram_tensor("xT", [d, m], F32, kind="ExternalInput").ap()
    wsh1 = nc.dram_tensor("wsh1", [d, shw], F32, kind="ExternalInput").ap()
    wsh3 = nc.dram_tensor("wsh3", [d, shw], F32, kind="ExternalInput").ap()
    w0r1 = nc.dram_tensor("w0r1", [d, 8 * 128], F32, kind="ExternalInput").ap()
    w0r3 = nc.dram_tensor("w0r3", [d, 8 * 128], F32, kind="ExternalInput").ap()
    wsh2 = nc.dram_tensor("wsh2", [128, sw], F32, kind="ExternalInput").ap()
    y = nc.dram_tensor("y", [m, d], F32, kind="ExternalOutput").ap()

    xview = xT.rearrange("(k p) m -> p k m", p=128)
    v1 = wsh1.rearrange("(k p) h -> p k h", p=128)    # [128, 16, 768]
    v3 = wsh3.rearrange("(k p) h -> p k h", p=128)
    r1 = w0r1.rearrange("(k p) h -> p k h", p=128)    # [128, 16, 1024]
    r3 = w0r3.rearrange("(k p) h -> p k h", p=128)

    with tile.TileContext(nc) as tc:
        with ExitStack() as ctx:
            dram = ctx.enter_context(tc.tile_pool(name="dram", bufs=1, space="DRAM"))
            # g in processed-tile order, split so phase C loads start mid-B
            g_a = dram.tile([N_GA * 128, m], BF16, tag="g_a", name="g_a")
            g_b = dram.tile([(ht - N_GA) * 128, m], BF16, tag="g_b", name="g_b")
            g_a_rd = g_a.rearrange("(q p) m -> p q m", p=128)
            g_b_rd = g_b.rearrange("(q p) m -> p q m", p=128)
            cc_in = dram.tile([1, 8], F32, tag="cc_in", name="cc_in")
            cc_out = dram.tile([1, 8], F32, tag="cc_out", name="cc_out")
            # w13 AllGather (slots 1..5 of every core)
            agin = dram.tile([d, 2, (SLOTS - 1) * 128], FP8, tag="agi", name="agi")
            agout = dram.tile([n_cores * d, 2, (SLOTS - 1) * 128], FP8,
                              tag="ago", name="ago")
            agin_v = agin.rearrange("(k p) two h -> p k two h", p=128)
            agout_v = agout.rearrange("(j k p) two h -> p j k two h",
                                      p=128, k=kd)
            # w2 AllGather (baseline scheme)
            agin2 = dram.tile([128, sw], FP8, tag="agi2", name="agi2")
            agout2 = dram.tile([HP, d], FP8, tag="ago2", name="ago2")
            agout2_rd = agout2.rearrange("(k p) dd -> p k dd", p=128)

            # ---- persistent SBUF ------------------------------------------
            pc = ctx.enter_context(tc.tile_pool(name="pconst", bufs=1))
            w2a_p = ctx.enter_context(tc.tile_pool(name="w2a", bufs=1))
            w2A = w2a_p.tile([128, ht, 1024], FP8, tag="w2A", name="w2A")

            # ---------------- warm-up stream (PE FIFO head) -----------------
            warm = pc.tile([128, 128], BF16, tag="warm", name="warm")
            wrhs = pc.tile([128, 512], BF16, tag="wrhs", name="wrhs")
            nc.vector.memset(warm, 0.125)
            nc.vector.memset(wrhs, 0.25)
            with tc.tile_pool(name="wps", bufs=1, space="PSUM") as wps:
                wz = wps.tile([128, 512], F32, tag="wz", name="wz")
                for i in range(N_WARM):
                    nc.tensor.matmul(wz, lhsT=warm, rhs=wrhs,
                                     start=(i == 0), stop=(i == N_WARM - 1))

            bias = {}
            scl = {}
            with ExitStack() as sab:           # phase A + ternarize + phase B
                xp = sab.enter_context(tc.tile_pool(name="xp", bufs=1))
                x_sb = xp.tile([128, kd, m], BF16, tag="x_sb", name="x_sb")
                qb_p = sab.enter_context(tc.tile_pool(name="qbp", bufs=2))
                rep_p = sab.enter_context(tc.tile_pool(name="rep", bufs=8))

                # 2-op DVE ternarize to exact {-1,0,+1} fp8:
                #   qb = (w < bn)  ;  qt = (w >= bp) - qb
                def tern(dst, src, tname, pool=None):
                    qb = (pool or qb_p).tile(list(src.shape), FP8,
                                             tag=f"qb{src.shape[-1]}", name="qb")
                    nc.vector.tensor_scalar(qb, src, bias[tname + "n"],
                                            None, ALU.is_lt)
                    nc.vector.scalar_tensor_tensor(
                        dst, src, bias[tname + "p"], qb,
                        ALU.is_ge, ALU.subtract)

                rep_wq = {}

                with ExitStack() as sa:        # startup-only staging
                    sc_pool = sa.enter_context(tc.tile_pool(name="scale", bufs=2))
                    sc1 = sa.enter_context(tc.tile_pool(name="scale1", bufs=1))
                    rr_p = sa.enter_context(tc.tile_pool(name="rr", bufs=2))
                    qtp = sa.enter_context(tc.tile_pool(name="qtp", bufs=2))

                    # -------- phase A: scales ------------------------------
                    asum = sc1.tile([128, 3, 8], F32, tag="asum", name="asum")
                    nc.vector.memset(asum, 0.0)
                    for t, v in ((0, v1), (1, v3)):
                        for s in range(SLOTS):
                            hs_ = slice(s * 128, (s + 1) * 128)
                            stg = sc_pool.tile([128, kd, 128], F32,
                                               tag="scstg", name="scstg")
                            nc.sync.dma_start(stg, v[:, :, hs_])
                            nc.vector.tensor_reduce(
                                asum[:, t, s:s + 1], stg, axis=AX.XY,
                                op=ALU.add, apply_absolute_value=True,
                            )
                    chunk2 = sw // 8
                    for c in range(8):
                        cs = slice(c * chunk2, (c + 1) * chunk2)
                        stg = sc_pool.tile([128, chunk2], F32, tag="sc2stg",
                                           name="sc2stg")
                        nc.sync.dma_start(stg, wsh2[:, cs])
                        nc.vector.tensor_reduce(
                            asum[:, 2, c:c + 1], stg, axis=AX.XY, op=ALU.add,
                            apply_absolute_value=True,
                        )
                    part8 = sc1.tile([128, 8], F32, tag="part8", name="part8")
                    nc.vector.memset(part8, 0.0)
                    nc.vector.tensor_reduce(part8[:, 0:3], asum, axis=AX.X,
                                            op=ALU.add)
                    pall = sc1.tile([128, 8], F32, tag="pall", name="pall")
                    nc.gpsimd.partition_all_reduce(pall, part8, 128,
                                                   bass_isa.ReduceOp.add)
                    nc.gpsimd.dma_start(cc_in, pall[0:1, :])
                    nc.gpsimd.collective_compute(
                        "AllReduce",
                        ALU.add,
                        replica_groups=[list(range(n_cores))],
                        ins=[cc_in.opt()],
                        outs=[cc_out.opt()],
                    )
                    g8 = sc1.tile([1, 8], F32, tag="g8", name="g8")
                    nc.gpsimd.dma_start(g8, cc_out)
                    gb = pc.tile([128, 8], F32, tag="gb", name="gb")
                    nc.gpsimd.partition_broadcast(gb, g8)
                    for t, name in enumerate(["w1", "w3", "w2"]):
                        for sgn in ("p", "n"):
                            bias[name + sgn] = pc.tile(
                                [128, 1], F32, tag=f"b_{name}{sgn}",
                                name=f"b_{name}{sgn}")
                            k = 0.5 / n_true if sgn == "p" else -0.5 / n_true
                            nc.vector.tensor_scalar(
                                bias[name + sgn], gb[:, t:t + 1], k, None,
                                ALU.mult,
                            )
                    # epilogue scales for {-1,0,1} ternary values
                    for t, name in enumerate(["w1", "w3", "w2"]):
                        scl[name] = pc.tile([128, 1], F32, tag=f"s_{name}",
                                            name=f"s_{name}")
                        nc.vector.tensor_scalar(
                            scl[name], gb[:, t:t + 1], 1.0 / n_true, None,
                            ALU.mult)
                    s23 = pc.tile([128, 1], F32, tag="s23", name="s23")
                    nc.vector.tensor_mul(s23, scl["w3"], scl["w2"])

                    # x loads: deferred behind phase A's DMA on the Act queue
                    # via a dummy Act op reading the last phase-A staging tile.
                    xgate = pc.tile([128, 1], F32, tag="xgate", name="xgate")
                    nc.scalar.copy(xgate, stg[:, 0:1])
                    with tc.tile_pool(name="xstg", bufs=2) as xstg_p:
                        for k in range(kd):
                            for hlf in range(2):
                                msl = slice(hlf * (m // 2), (hlf + 1) * (m // 2))
                                xstg = xstg_p.tile([128, m // 2], F32,
                                                   tag="xstg", name="xstg")
                                nc.scalar.dma_start(xstg, xview[:, k, msl])
                                nc.scalar.copy(x_sb[:, k, msl], xstg)

                    # -------- ternarize w1/w3 ------------------------------
                    def tern_rep(j):
                        rt = rep_p.tile([128, kd, 2, 128], FP8, tag="rept",
                                        name="rept")
                        for t, rv in ((0, r1), (1, r3)):
                            hs_ = slice(j * 128, (j + 1) * 128)
                            stg = rr_p.tile([128, kd, 128], F32, tag="rstg",
                                            name="rstg")
                            nc.sync.dma_start(stg, rv[:, :, hs_])
                            tern(rt[:, :, t, :], stg,
                                 "w1" if t == 0 else "w3", qtp)
                        rep_wq[j] = rt

                    def tern_own(t, s):
                        tname = "w1" if t == 0 else "w3"
                        src = rr_p.tile([128, kd, 128], F32, tag="rstg",
                                        name="rstg")
                        v = v1 if t == 0 else v3
                        nc.sync.dma_start(
                            src, v[:, :, s * 128:(s + 1) * 128])
                        qt = qtp.tile([128, kd, 128], FP8, tag="qt",
                                      name="qt")
                        tern(qt, src, tname, qtp)
                        nc.sync.dma_start(
                            agin_v[:, :, t, (s - 1) * 128:s * 128], qt)

                    # order: all rep tiles first (direct phase-B runway),
                    # then own slots (the AllGather payload; its trigger is
                    # emitted inside phase B to keep the GPSIMD queue clean).
                    for j in range(n_cores):
                        tern_rep(j)
                    for s in range(1, SLOTS):
                        for t in range(2):
                            tern_own(t, s)

                # ------------- phase B -------------------------------------
                wq_p = sab.enter_context(tc.tile_pool(name="wq", bufs=3))
                sl_p = sab.enter_context(tc.tile_pool(name="slp", bufs=3))
                scp_p = sab.enter_context(tc.tile_pool(name="scp", bufs=3))
                g_p = sab.enter_context(tc.tile_pool(name="gp", bufs=2))
                q2s_p = sab.enter_context(tc.tile_pool(name="q2s", bufs=2))
                zps = sab.enter_context(
                    tc.tile_pool(name="zps", bufs=8, space="PSUM"))

                # deferred w2 ternarize/AllGather/prefetch, paced into B
                n2 = 16
                c2 = sw // n2

                def emit_q2_piece(c):
                    cs = slice(c * c2, (c + 1) * c2)
                    stg = q2s_p.tile([128, c2], F32, tag="q2stg",
                                     name="q2stg")
                    nc.sync.dma_start(stg, wsh2[:, cs])
                    qt = q2s_p.tile([128, c2], FP8, tag="q2t", name="q2t")
                    tern(qt, stg, "w2")
                    nc.sync.dma_start(agin2[:, cs], qt)

                pending = [("q2", c) for c in range(n2)] + [("ag2", None)] + \
                          [("w2a", k2) for k2 in range(ht)]
                pend_i = 0

                def drain_pending(n):
                    nonlocal pend_i
                    for _ in range(n):
                        if pend_i >= len(pending):
                            return
                        kind, pl = pending[pend_i]
                        pend_i += 1
                        if kind == "q2":
                            emit_q2_piece(pl)
                        elif kind == "ag2":
                            nc.gpsimd.collective_compute(
                                "AllGather", ALU.bypass,
                                replica_groups=[list(range(n_cores))],
                                ins=[agin2.opt()],
                                outs=[agout2.opt()],
                            )
                        elif kind == "w2a":
                            nc.gpsimd.dma_start(
                                w2A[:, pl, :], agout2_rd[:, pl, 0:1024])

                # weight sub-blocks: (core j, slots s0..s0+ns-1), ns <= 2
                SUBS = []
                for j in range(n_cores):
                    s = 1
                    while s < OWN_N[j]:
                        ns = min(2, OWN_N[j] - s)
                        SUBS.append((j, s, ns))
                        s += ns

                def load_sub(i):
                    j, s0, ns = SUBS[i]
                    wq = wq_p.tile([128, kd, 2, 256], FP8,
                                   tag="wqb", name="wqb")
                    for t in range(2):
                        nc.sync.dma_start(
                            wq[:, :, t, :ns * 128],
                            agout_v[:, j, :, t,
                                    (s0 - 1) * 128:(s0 - 1 + ns) * 128])
                    return wq

                def do_tile(proc_i, wq_ap):
                    """wq_ap: [128, kd, 2, >=128] fp8 for this tile."""
                    zz = [[None] * mc, [None] * mc]
                    for t in range(2):
                        for mci in range(mc):
                            zz[t][mci] = zps.tile([128, 512], F32,
                                                  tag="z", name="z")
                        for k in range(kd):
                            for mci in range(mc):
                                nc.tensor.matmul(
                                    zz[t][mci], lhsT=wq_ap[:, k, t, 0:128],
                                    rhs=x_sb[:, k,
                                             mci * 512:(mci + 1) * 512],
                                    start=(k == 0), stop=(k == kd - 1),
                                )
                    g_t = g_p.tile([128, m], BF16, tag="g_t", name="g_t")
                    for mci in range(mc):
                        ms = slice(mci * 512, (mci + 1) * 512)
                        sl = sl_p.tile([128, 512], BF16, tag="sl", name="sl")
                        nc.scalar.activation(sl, zz[0][mci], AF.Silu,
                                             bias=0.0, scale=scl["w1"])
                        sc = scp_p.tile([128, 512], BF16, tag="sc", name="sc")
                        nc.scalar.activation(sc, zz[1][mci], AF.Copy,
                                             bias=0.0, scale=s23)
                        nc.gpsimd.tensor_mul(g_t[:, ms], sl, sc)
                    if proc_i < N_GA:
                        gdst = g_a[proc_i * 128:(proc_i + 1) * 128, :]
                    else:
                        gdst = g_b[(proc_i - N_GA) * 128:
                                   (proc_i - N_GA + 1) * 128, :]
                    nc.scalar.dma_start(gdst, g_t)

                # slot-0 tiles (locally ternarized, zero collective latency)
                for j in range(n_cores):
                    drain_pending(1)
                    do_tile(j, rep_wq.pop(j))
                    if j == 2:
                        nc.gpsimd.collective_compute(
                            "AllGather", ALU.bypass,
                            replica_groups=[list(range(n_cores))],
                            ins=[agin.opt()],
                            outs=[agout.opt()],
                        )

                # remaining tiles, 2-tile sub-blocks from the AllGather
                blk = load_sub(0)
                nxt = load_sub(1)
                proc_i = n_cores
                for i, (j, s0, ns) in enumerate(SUBS):
                    for si in range(ns):
                        drain_pending(2)
                        do_tile(proc_i, blk[:, :, :, si * 128:(si + 1) * 128])
                        proc_i += 1
                    blk = nxt
                    nxt = load_sub(i + 2) if i + 2 < len(SUBS) else None
                drain_pending(len(pending))
                assert proc_i == ht

            # ---------------- phase C (d in two halves) ---------------------
            with ExitStack() as scx:
                w2b_p = scx.enter_context(tc.tile_pool(name="w2b", bufs=1))
                gq_p = scx.enter_context(tc.tile_pool(name="gq", bufs=3))
                y_p = scx.enter_context(tc.tile_pool(name="yp", bufs=4))
                yps = scx.enter_context(
                    tc.tile_pool(name="yps", bufs=8, space="PSUM"))

                gq_tiles = {}

                def emit_gq(mt):
                    gq = gq_p.tile([128, ht, 128], BF16, tag="gq", name="gq")
                    ms = slice(mt * 128, (mt + 1) * 128)
                    nc.sync.dma_start(gq[:, 0:N_GA, :], g_a_rd[:, :, ms])
                    nc.sync.dma_start(gq[:, N_GA:ht, :], g_b_rd[:, :, ms])
                    gq_tiles[mt] = gq

                emit_gq(0)
                w2B = w2b_p.tile([128, ht, 1024], FP8, tag="w2B", name="w2B")
                for k2 in range(ht):
                    nc.sync.dma_start(w2B[:, k2, :],
                                      agout2_rd[:, k2, 1024:2048])

                w2h = [w2A, w2B]
                for mt in range(m // 128):
                    if mt not in gq_tiles:
                        emit_gq(mt)
                    gq = gq_tiles.pop(mt)
                    for half in range(2):
                        yp2 = [yps.tile([128, 512], F32, tag="yps",
                                        name="yps")
                               for _ in range(2)]
                        for k2 in range(ht):
                            a_t = PROC[k2]
                            for di in range(2):
                                nc.tensor.matmul(
                                    yp2[di],
                                    lhsT=gq[:, k2, :],
                                    rhs=w2h[half][:, a_t,
                                                  di * 512:(di + 1) * 512],
                                    start=(k2 == 0), stop=(k2 == ht - 1),
                                )
                        ysb = y_p.tile([128, 1024], F32, tag="ysb",
                                       name="ysb")
                        for di in range(2):
                            nc.scalar.copy(ysb[:, di * 512:(di + 1) * 512],
                                           yp2[di])
                        nc.scalar.dma_start(
                            y[mt * 128:(mt + 1) * 128,
                              half * 1024:(half + 1) * 1024], ysb)

    nc.compile()
    return nc


_NC_CACHE = {}


def _get_module():
    if "nc" not in _NC_CACHE:
        _NC_CACHE["nc"] = build_module()
    return _NC_CACHE["nc"]


def prep_inputs(x, w1, w3, w2, d=D, m=M, n_cores=N_CORES):
    """Host-side layout work: pad, transpose, shard, slice. No arithmetic."""
    h_real = w1.shape[0]
    x = np.ascontiguousarray(np.asarray(x, dtype=np.float32))
    xf = x.reshape(-1, d)
    w1t = np.zeros((d, HP), np.float32)
    w1t[:, :h_real] = np.asarray(w1, np.float32).T
    w3t = np.zeros((d, HP), np.float32)
    w3t[:, :h_real] = np.asarray(w3, np.float32).T
    w2t = np.zeros((HP, d), np.float32)
    w2t[:h_real, :] = np.asarray(w2, np.float32).T

    r2 = HP // n_cores
    sw = d * HP // (n_cores * 128)
    shw = SLOTS * 128

    # slot-0 tiles of every core, replicated
    s0_1 = np.ascontiguousarray(np.concatenate(
        [w1t[:, OWN_START[j] * 128:(OWN_START[j] + 1) * 128]
         for j in range(n_cores)], axis=1))
    s0_3 = np.ascontiguousarray(np.concatenate(
        [w3t[:, OWN_START[j] * 128:(OWN_START[j] + 1) * 128]
         for j in range(n_cores)], axis=1))

    in_maps = []
    for c in range(n_cores):
        xc = np.ascontiguousarray(xf[c * m:(c + 1) * m].T)   # [d, m]
        wsh1 = np.zeros((d, shw), np.float32)
        wsh3 = np.zeros((d, shw), np.float32)
        t0 = OWN_START[c] * 128
        nh = OWN_N[c] * 128
        wsh1[:, :nh] = w1t[:, t0:t0 + nh]
        wsh3[:, :nh] = w3t[:, t0:t0 + nh]
        in_maps.append({
            "xT": xc,
            "wsh1": wsh1,
            "wsh3": wsh3,
            "w0r1": s0_1,
            "w0r3": s0_3,
            "wsh2": np.ascontiguousarray(
                w2t[c * r2:(c + 1) * r2].reshape(128, sw)),
        })
    return in_maps


def kernel(x, w1, w3, w2):
    from concourse.bass_utils import run_bass_kernel_spmd

    nc = _get_module()
    in_maps = prep_inputs(x, w1, w3, w2)
    res = run_bass_kernel_spmd(nc, in_maps, core_ids=list(range(N_CORES)))
    _NC_CACHE["last_results"] = res
    yf = np.concatenate([r["y"] for r in res.results], axis=0)  # [16384, 2048]
    return np.ascontiguousarray(yf.reshape(B, S, D).astype(np.float32))


# revision 12
# speedup vs baseline: 1.0667x; 1.0667x over previous
"""Trainium2 Bass kernel for nn_BinaryMLP (BitNet-ternary SwiGLU MLP).

reference math (fp32):
    s_i = mean(|w_i|)            (per-tensor scalar, i in {1,3,2})
    wq_i = clip(round(w_i/s_i), -1, 1) * s_i     (ternary * scale)
    h1 = x @ w1q.T ; h3 = x @ w3q.T
    y  = (silu(h1) * h3) @ w2q.T

Strategy (8 cores, data-parallel over the 16384 tokens).  Per core:
  phase A: |w| partial sums over this core's h-column shard of w1/w3 and
           d-row shard of w2 (DVE abs-reduce, GPSIMD partition reduce) ->
           tiny 8-core AllReduce -> per-partition ternarize thresholds.
  tern:    w1/w3 are sharded across cores by h-TILES (core j owns tiles
           own_start[j]..+own_n[j]-1).  Slot-0 tiles of every core are
           additionally replicated in fp32 to every core, so each core
           ternarizes tiles {own_start[j]} locally and phase B starts
           with 8 tiles of runway before the single AllGather (slots
           1..5 of every core) has to land.  Ternary values are exact
           {-1,0,+1} in fp8; per-tensor scales fold into the epilogue.
  phase B: h-tile loop over the 43 tiles in a fixed processed order
           (slot-0 tiles first), each tile = 128 h rows x 2048 tokens:
           z1/z3 matmuls vs resident bf16 x (fp8 ternary weights
           stationary, 8 psum banks), epilogue split across engines so
           nothing head-of-line blocks: silu on Act, z3 scaling on Act
           (Copy+scale), the g product on GPSIMD, ternarize only on DVE.
           g goes to DRAM in processed-tile order, split into two
           tensors so phase C's first token-tile loads start mid-B.
           w2 is ternarized+AllGathered and half of it prefetched to
           SBUF during B (baseline scheme).
  phase C: y[m,d] = sum_h g[h,m] w2q[h,d], g stationary, fp32 PSUM, d in
           two halves (half A SBUF-resident early, half B streamed).
  A long calibrated warm-up matmul stream keeps the PE HAM-warm through
  phase A so real matmuls start at full clock.

All arithmetic (scales, ternarization, matmuls) happens on device; the
host only reshapes / transposes / pads / slices / concatenates.
"""

import sys
from contextlib import ExitStack

import numpy as np

if "/opt/trn_rl_repo" not in sys.path:
    sys.path.insert(0, "/opt/trn_rl_repo")

import concourse.bass as bass  # noqa: E402,F401
import concourse.bass_isa as bass_isa  # noqa: E402
import concourse.mybir as mybir  # noqa: E402
import concourse.tile as tile  # noqa: E402
from concourse import bacc  # noqa: E402

F32 = mybir.dt.float32
BF16 = mybir.dt.bfloat16
FP8 = mybir.dt.float8e4
AF = mybir.ActivationFunctionType
ALU = mybir.AluOpType
AX = mybir.AxisListType

# Full problem geometry (hardcoded per contest rules).
B, S, D = 4, 4096, 2048
H_REAL = 5461
HP = 5504            # H padded to 43*128
N_CORES = 8
M = (B * S) // N_CORES   # tokens per core = 2048
HT = HP // 128           # 43 h-tiles
KD = D // 128            # 16 k-tiles over D
MC = M // 512            # 4 moving chunks of 512 tokens

# h-tile ownership (global, same table baked into every core's program)
OWN_N = [5, 5, 5, 5, 5, 6, 6, 6]
OWN_START = [0, 5, 10, 15, 20, 25, 31, 37]
assert sum(OWN_N) == HT
SLOTS = 6                 # padded slots per core (zero-pad for own_n=5)

# processed-tile order: slot-0 tiles of all cores first (locally
# ternarized from the replicated input -> zero collective latency),
# then slots 1..5 per core (delivered by the one AllGather).
PROC = [OWN_START[j] for j in range(N_CORES)] + [
    OWN_START[j] + s
    for j in range(N_CORES)
    for s in (1, 2)
] + [
    OWN_START[j] + s
    for j in range(N_CORES)
    for s in (3, 4)
    if s < OWN_N[j]
] + [
    OWN_START[j] + 5
    for j in range(N_CORES)
    if OWN_N[j] == 6
]
assert sorted(PROC) == list(range(HT))
N_GA = 22                 # processed tiles 0..21 -> g_a, rest -> g_b
N_WARM = 340              # warm-up matmuls (keeps PE busy through phase A)


def build_module(d=D, m=M, n_cores=N_CORES, h_real=H_REAL):
    kd, ht, mc = KD, HT, MC
    n_true = h_real * d
    sw = d * HP // (n_cores * 128)   # w2 slice free elems per partition
    shw = SLOTS * 128                # 768 padded own h-cols of w1/w3

    nc = bacc.Bacc(
        "TRN2",
        target_bir_lowering=False,
        debug=False,
        num_devices=n_cores,
    )
    xT = nc.dram_tensor("xT", [d, m], F32, kind="ExternalInput").ap()
    wsh1 = nc.dram_tensor("wsh1", [d, shw], F32, kind="ExternalInput").ap()
    wsh3 = nc.dram_tensor("wsh3", [d, shw], F32, kind="ExternalInput").ap()
    w0r1 = nc.dram_tensor("w0r1", [d, 8 * 128], F32, kind="ExternalInput").ap()
    w0r3 = nc.dram_tensor("w0r3", [d, 8 * 128], F32, kind="ExternalInput").ap()
    wsh2 = nc.dram_tensor("wsh2", [128, sw], F32, kind="ExternalInput").ap()
    y = nc.dram_tensor("y", [m, d], F32, kind="ExternalOutput").ap()

    xview = xT.rearrange("(k p) m -> p k m", p=128)
    v1 = wsh1.rearrange("(k p) h -> p k h", p=128)    # [128, 16, 768]
    v3 = wsh3.rearrange("(k p) h -> p k h", p=128)
    r1 = w0r1.rearrange("(k p) h -> p k h", p=128)    # [128, 16, 1024]
    r3 = w0r3.rearrange("(k p) h -> p k h", p=128)

    with tile.TileContext(nc) as tc:
        with ExitStack() as ctx:
            dram = ctx.enter_context(tc.tile_pool(name="dram", bufs=1, space="DRAM"))
            # g in processed-tile order, split so phase C loads start mid-B
            g_a = dram.tile([N_GA * 128, m], BF16, tag="g_a", name="g_a")
            g_b = dram.tile([(ht - N_GA) * 128, m], BF16, tag="g_b", name="g_b")
            g_a_rd = g_a.rearrange("(q p) m -> p q m", p=128)
            g_b_rd = g_b.rearrange("(q p) m -> p q m", p=128)
            cc_in = dram.tile([1, 8], F32, tag="cc_in", name="cc_in")
            cc_out = dram.tile([1, 8], F32, tag="cc_out", name="cc_out")
            # w13 AllGathers (a: slots 1-2, b: slots 3-5 of every core)
            agin_a = dram.tile([d, 2, 256], FP8, tag="agia", name="agia")
            agout_a = dram.tile([n_cores * d, 2, 256], FP8,
                                tag="agoa", name="agoa")
            agin_b = dram.tile([d, 2, 384], FP8, tag="agib", name="agib")
            agout_b = dram.tile([n_cores * d, 2, 384], FP8,
                                tag="agob", name="agob")
            agin_av = agin_a.rearrange("(k p) two h -> p k two h", p=128)
            agin_bv = agin_b.rearrange("(k p) two h -> p k two h", p=128)
            agout_av = agout_a.rearrange("(j k p) two h -> p j k two h",
                                         p=128, k=kd)
            agout_bv = agout_b.rearrange("(j k p) two h -> p j k two h",
                                         p=128, k=kd)
            # w2 AllGather (baseline scheme)
            agin2 = dram.tile([128, sw], FP8, tag="agi2", name="agi2")
            agout2 = dram.tile([HP, d], FP8, tag="ago2", name="ago2")
            agout2_rd = agout2.rearrange("(k p) dd -> p k dd", p=128)

            # ---- persistent SBUF ------------------------------------------
            pc = ctx.enter_context(tc.tile_pool(name="pconst", bufs=1))
            w2a_p = ctx.enter_context(tc.tile_pool(name="w2a", bufs=1))
            w2A = w2a_p.tile([128, ht, 1024], FP8, tag="w2A", name="w2A")

            # ---------------- warm-up stream (PE FIFO head) -----------------
            warm = pc.tile([128, 128], BF16, tag="warm", name="warm")
            wrhs = pc.tile([128, 512], BF16, tag="wrhs", name="wrhs")
            nc.vector.memset(warm, 0.125)
            nc.vector.memset(wrhs, 0.25)
            with tc.tile_pool(name="wps", bufs=1, space="PSUM") as wps:
                wz = wps.tile([128, 512], F32, tag="wz", name="wz")
                for i in range(N_WARM):
                    nc.tensor.matmul(wz, lhsT=warm, rhs=wrhs,
                                     start=(i == 0), stop=(i == N_WARM - 1))

            bias = {}
            scl = {}
            with ExitStack() as sab:           # phase A + ternarize + phase B
                xp = sab.enter_context(tc.tile_pool(name="xp", bufs=1))
                x_sb = xp.tile([128, kd, m], BF16, tag="x_sb", name="x_sb")
                qb_p = sab.enter_context(tc.tile_pool(name="qbp", bufs=2))
                rep_p = sab.enter_context(tc.tile_pool(name="rep", bufs=8))

                # 2-op DVE ternarize to exact {-1,0,+1} fp8:
                #   qb = (w < bn)  ;  qt = (w >= bp) - qb
                def tern(dst, src, tname, pool=None):
                    qb = (pool or qb_p).tile(list(src.shape), FP8,
                                             tag=f"qb{src.shape[-1]}", name="qb")
                    nc.vector.tensor_scalar(qb, src, bias[tname + "n"],
                                            None, ALU.is_lt)
                    nc.vector.scalar_tensor_tensor(
                        dst, src, bias[tname + "p"], qb,
                        ALU.is_ge, ALU.subtract)

                rep_wq = {}

                with ExitStack() as sa:        # startup-only staging
                    sc_pool = sa.enter_context(tc.tile_pool(name="scale", bufs=2))
                    sc1 = sa.enter_context(tc.tile_pool(name="scale1", bufs=1))
                    rr_p = sa.enter_context(tc.tile_pool(name="rr", bufs=3))
                    qtp = sa.enter_context(tc.tile_pool(name="qtp", bufs=2))

                    # -------- phase A: scales ------------------------------
                    asum = sc1.tile([128, 3, 8], F32, tag="asum", name="asum")
                    nc.vector.memset(asum, 0.0)
                    for t, v in ((0, v1), (1, v3)):
                        for s in range(SLOTS):
                            hs_ = slice(s * 128, (s + 1) * 128)
                            stg = sc_pool.tile([128, kd, 128], F32,
                                               tag="scstg", name="scstg")
                            nc.sync.dma_start(stg, v[:, :, hs_])
                            nc.vector.tensor_reduce(
                                asum[:, t, s:s + 1], stg, axis=AX.XY,
                                op=ALU.add, apply_absolute_value=True,
                            )
                    chunk2 = sw // 8
                    for c in range(8):
                        cs = slice(c * chunk2, (c + 1) * chunk2)
                        stg = sc_pool.tile([128, chunk2], F32, tag="sc2stg",
                                           name="sc2stg")
                        nc.sync.dma_start(stg, wsh2[:, cs])
                        nc.vector.tensor_reduce(
                            asum[:, 2, c:c + 1], stg, axis=AX.XY, op=ALU.add,
                            apply_absolute_value=True,
                        )
                    part8 = sc1.tile([128, 8], F32, tag="part8", name="part8")
                    nc.vector.memset(part8, 0.0)
                    nc.vector.tensor_reduce(part8[:, 0:3], asum, axis=AX.X,
                                            op=ALU.add)
                    pall = sc1.tile([128, 8], F32, tag="pall", name="pall")
                    nc.gpsimd.partition_all_reduce(pall, part8, 128,
                                                   bass_isa.ReduceOp.add)
                    nc.gpsimd.dma_start(cc_in, pall[0:1, :])
                    nc.gpsimd.collective_compute(
                        "AllReduce",
                        ALU.add,
                        replica_groups=[list(range(n_cores))],
                        ins=[cc_in.opt()],
                        outs=[cc_out.opt()],
                    )
                    g8 = sc1.tile([1, 8], F32, tag="g8", name="g8")
                    nc.gpsimd.dma_start(g8, cc_out)
                    gb = pc.tile([128, 8], F32, tag="gb", name="gb")
                    nc.gpsimd.partition_broadcast(gb, g8)
                    for t, name in enumerate(["w1", "w3", "w2"]):
                        for sgn in ("p", "n"):
                            bias[name + sgn] = pc.tile(
                                [128, 1], F32, tag=f"b_{name}{sgn}",
                                name=f"b_{name}{sgn}")
                            k = 0.5 / n_true if sgn == "p" else -0.5 / n_true
                            nc.vector.tensor_scalar(
                                bias[name + sgn], gb[:, t:t + 1], k, None,
                                ALU.mult,
                            )
                    # epilogue scales for {-1,0,1} ternary values
                    for t, name in enumerate(["w1", "w3", "w2"]):
                        scl[name] = pc.tile([128, 1], F32, tag=f"s_{name}",
                                            name=f"s_{name}")
                        nc.vector.tensor_scalar(
                            scl[name], gb[:, t:t + 1], 1.0 / n_true, None,
                            ALU.mult)
                    s23 = pc.tile([128, 1], F32, tag="s23", name="s23")
                    nc.vector.tensor_mul(s23, scl["w3"], scl["w2"])

                    # x loads: staged through the SAME pool slots as the
                    # phase-A reduce staging (tag+shape match), so the pool
                    # rotation itself defers x behind phase A's reads.
                    for k in range(kd):
                        xstg = sc_pool.tile([128, kd, 128], F32,
                                            tag="scstg", name="scstg")
                        nc.scalar.dma_start(
                            xstg, xview[:, k, :].rearrange(
                                "p (a b) -> p a b", a=kd))
                        nc.scalar.copy(
                            x_sb[:, k, :],
                            xstg.rearrange("p a b -> p (a b)"))

                    # -------- ternarize w1/w3 ------------------------------
                    def tern_rep(j):
                        rt = rep_p.tile([128, kd, 2, 128], FP8, tag="rept",
                                        name="rept")
                        for t, rv in ((0, r1), (1, r3)):
                            hs_ = slice(j * 128, (j + 1) * 128)
                            stg = rr_p.tile([128, kd, 128], F32, tag="rstg",
                                            name="rstg")
                            nc.sync.dma_start(stg, rv[:, :, hs_])
                            tern(rt[:, :, t, :], stg,
                                 "w1" if t == 0 else "w3", qtp)
                        rep_wq[j] = rt

                    def tern_own(t, s):
                        tname = "w1" if t == 0 else "w3"
                        src = rr_p.tile([128, kd, 128], F32, tag="rstg",
                                        name="rstg")
                        v = v1 if t == 0 else v3
                        nc.sync.dma_start(
                            src, v[:, :, s * 128:(s + 1) * 128])
                        qt = qtp.tile([128, kd, 128], FP8, tag="qt",
                                      name="qt")
                        tern(qt, src, tname, qtp)
                        if s <= 2:
                            nc.sync.dma_start(
                                agin_av[:, :, t, (s - 1) * 128:s * 128], qt)
                        else:
                            nc.sync.dma_start(
                                agin_bv[:, :, t, (s - 3) * 128:(s - 2) * 128],
                                qt)

                    # order: all rep tiles first (direct phase-B runway),
                    # then own slots (the AllGather payload; its trigger is
                    # emitted inside phase B to keep the GPSIMD queue clean).
                    for j in range(n_cores):
                        tern_rep(j)
                    for s in (1, 2):
                        for t in range(2):
                            tern_own(t, s)
                    nc.gpsimd.collective_compute(
                        "AllGather", ALU.bypass,
                        replica_groups=[list(range(n_cores))],
                        ins=[agin_a.opt()],
                        outs=[agout_a.opt()],
                    )
                    for s in (3, 4, 5):
                        for t in range(2):
                            tern_own(t, s)
                    nc.gpsimd.collective_compute(
                        "AllGather", ALU.bypass,
                        replica_groups=[list(range(n_cores))],
                        ins=[agin_b.opt()],
                        outs=[agout_b.opt()],
                    )

                # ------------- phase B -------------------------------------
                wq_p = sab.enter_context(tc.tile_pool(name="wq", bufs=2))
                sl_p = sab.enter_context(tc.tile_pool(name="slp", bufs=3))
                scp_p = sab.enter_context(tc.tile_pool(name="scp", bufs=3))
                g_p = sab.enter_context(tc.tile_pool(name="gp", bufs=2))
                q2s_p = sab.enter_context(tc.tile_pool(name="q2s", bufs=2))
                zps = sab.enter_context(
                    tc.tile_pool(name="zps", bufs=8, space="PSUM"))

                # deferred w2 ternarize/AllGather/prefetch, paced into B
                n2 = 16
                c2 = sw // n2

                def emit_q2_piece(c):
                    cs = slice(c * c2, (c + 1) * c2)
                    stg = q2s_p.tile([128, c2], F32, tag="q2stg",
                                     name="q2stg")
                    nc.sync.dma_start(stg, wsh2[:, cs])
                    qt = q2s_p.tile([128, c2], FP8, tag="q2t", name="q2t")
                    tern(qt, stg, "w2")
                    nc.sync.dma_start(agin2[:, cs], qt)

                pending = [("q2", c) for c in range(n2)] + [("ag2", None)] + \
                          [("w2a", k2) for k2 in range(ht)]
                pend_i = 0

                def drain_pending(n):
                    nonlocal pend_i
                    for _ in range(n):
                        if pend_i >= len(pending):
                            return
                        kind, pl = pending[pend_i]
                        pend_i += 1
                        if kind == "q2":
                            emit_q2_piece(pl)
                        elif kind == "ag2":
                            nc.gpsimd.collective_compute(
                                "AllGather", ALU.bypass,
                                replica_groups=[list(range(n_cores))],
                                ins=[agin2.opt()],
                                outs=[agout2.opt()],
                            )
                        elif kind == "w2a":
                            nc.gpsimd.dma_start(
                                w2A[:, pl, :], agout2_rd[:, pl, 0:1024])

                # weight sub-blocks: a-phase (j, slots 1-2) then b-phase
                # (j, slots 3..own_n-1), matching PROC order
                SUBS = [(j, 1, 2, 0) for j in range(n_cores)] + [
                    (j, 3, min(2, OWN_N[j] - 3), 1)
                    for j in range(n_cores)] + [
                    (j, 5, 1, 1) for j in range(n_cores) if OWN_N[j] == 6]

                def load_sub(i):
                    j, s0, ns, phase = SUBS[i]
                    wq = wq_p.tile([128, kd, 2, 256], FP8,
                                   tag="wqb", name="wqb")
                    agov = agout_av if phase == 0 else agout_bv
                    base = (s0 - 1) * 128 if phase == 0 else (s0 - 3) * 128
                    for t in range(2):
                        nc.sync.dma_start(
                            wq[:, :, t, :ns * 128],
                            agov[:, j, :, t, base:base + ns * 128])
                    return wq

                def do_tile(proc_i, wq_ap):
                    """wq_ap: [128, kd, 2, >=128] fp8 for this tile."""
                    zz = [[None] * mc, [None] * mc]
                    for t in range(2):
                        for mci in range(mc):
                            zz[t][mci] = zps.tile([128, 512], F32,
                                                  tag="z", name="z")
                        for k in range(kd):
                            for mci in range(mc):
                                nc.tensor.matmul(
                                    zz[t][mci], lhsT=wq_ap[:, k, t, 0:128],
                                    rhs=x_sb[:, k,
                                             mci * 512:(mci + 1) * 512],
                                    start=(k == 0), stop=(k == kd - 1),
                                )
                    g_t = g_p.tile([128, m], BF16, tag="g_t", name="g_t")
                    for mci in range(mc):
                        ms = slice(mci * 512, (mci + 1) * 512)
                        sl = sl_p.tile([128, 512], BF16, tag="sl", name="sl")
                        nc.scalar.activation(sl, zz[0][mci], AF.Silu,
                                             bias=0.0, scale=scl["w1"])
                        sc = scp_p.tile([128, 512], BF16, tag="sc", name="sc")
                        nc.scalar.activation(sc, zz[1][mci], AF.Copy,
                                             bias=0.0, scale=s23)
                        nc.gpsimd.tensor_mul(g_t[:, ms], sl, sc)
                    if proc_i < N_GA:
                        gdst = g_a[proc_i * 128:(proc_i + 1) * 128, :]
                    else:
                        gdst = g_b[(proc_i - N_GA) * 128:
                                   (proc_i - N_GA + 1) * 128, :]
                    nc.scalar.dma_start(gdst, g_t)

                # slot-0 tiles (locally ternarized, zero collective latency)
                for j in range(n_cores):
                    drain_pending(1)
                    do_tile(j, rep_wq.pop(j))

                # remaining tiles, 2-tile sub-blocks from the AllGather
                blk = load_sub(0)
                nxt = load_sub(1)
                proc_i = n_cores
                for i, (j, s0, ns, phase) in enumerate(SUBS):
                    for si in range(ns):
                        drain_pending(2)
                        do_tile(proc_i, blk[:, :, :, si * 128:(si + 1) * 128])
                        proc_i += 1
                    blk = nxt
                    nxt = load_sub(i + 2) if i + 2 < len(SUBS) else None
                drain_pending(len(pending))
                assert proc_i == ht

            # ---------------- phase C (d in two halves) ---------------------
            with ExitStack() as scx:
                w2b_p = scx.enter_context(tc.tile_pool(name="w2b", bufs=1))
                gq_p = scx.enter_context(tc.tile_pool(name="gq", bufs=3))
                y_p = scx.enter_context(tc.tile_pool(name="yp", bufs=4))
                yps = scx.enter_context(
                    tc.tile_pool(name="yps", bufs=8, space="PSUM"))

                gq_tiles = {}

                def emit_gq(mt):
                    gq = gq_p.tile([128, ht, 128], BF16, tag="gq", name="gq")
                    ms = slice(mt * 128, (mt + 1) * 128)
                    nc.sync.dma_start(gq[:, 0:N_GA, :], g_a_rd[:, :, ms])
                    nc.sync.dma_start(gq[:, N_GA:ht, :], g_b_rd[:, :, ms])
                    gq_tiles[mt] = gq

                emit_gq(0)
                w2B = w2b_p.tile([128, ht, 1024], FP8, tag="w2B", name="w2B")
                for k2 in range(ht):
                    nc.sync.dma_start(w2B[:, k2, :],
                                      agout2_rd[:, k2, 1024:2048])

                w2h = [w2A, w2B]
                for mt in range(m // 128):
                    if mt not in gq_tiles:
                        emit_gq(mt)
                    gq = gq_tiles.pop(mt)
                    for half in range(2):
                        yp2 = [yps.tile([128, 512], F32, tag="yps",
                                        name="yps")
                               for _ in range(2)]
                        for k2 in range(ht):
                            a_t = PROC[k2]
                            for di in range(2):
                                nc.tensor.matmul(
                                    yp2[di],
                                    lhsT=gq[:, k2, :],
                                    rhs=w2h[half][:, a_t,
                                                  di * 512:(di + 1) * 512],
                                    start=(k2 == 0), stop=(k2 == ht - 1),
                                )
                        ysb = y_p.tile([128, 1024], F32, tag="ysb",
                                       name="ysb")
                        for di in range(2):
                            nc.scalar.copy(ysb[:, di * 512:(di + 1) * 512],
                                           yp2[di])
                        nc.scalar.dma_start(
                            y[mt * 128:(mt + 1) * 128,
                              half * 1024:(half + 1) * 1024], ysb)

    nc.compile()
    return nc


_NC_CACHE = {}


def _get_module():
    if "nc" not in _NC_CACHE:
        _NC_CACHE["nc"] = build_module()
    return _NC_CACHE["nc"]


def prep_inputs(x, w1, w3, w2, d=D, m=M, n_cores=N_CORES):
    """Host-side layout work: pad, transpose, shard, slice. No arithmetic."""
    h_real = w1.shape[0]
    x = np.ascontiguousarray(np.asarray(x, dtype=np.float32))
    xf = x.reshape(-1, d)
    w1t = np.zeros((d, HP), np.float32)
    w1t[:, :h_real] = np.asarray(w1, np.float32).T
    w3t = np.zeros((d, HP), np.float32)
    w3t[:, :h_real] = np.asarray(w3, np.float32).T
    w2t = np.zeros((HP, d), np.float32)
    w2t[:h_real, :] = np.asarray(w2, np.float32).T

    r2 = HP // n_cores
    sw = d * HP // (n_cores * 128)
    shw = SLOTS * 128

    # slot-0 tiles of every core, replicated
    s0_1 = np.ascontiguousarray(np.concatenate(
        [w1t[:, OWN_START[j] * 128:(OWN_START[j] + 1) * 128]
         for j in range(n_cores)], axis=1))
    s0_3 = np.ascontiguousarray(np.concatenate(
        [w3t[:, OWN_START[j] * 128:(OWN_START[j] + 1) * 128]
         for j in range(n_cores)], axis=1))

    in_maps = []
    for c in range(n_cores):
        xc = np.ascontiguousarray(xf[c * m:(c + 1) * m].T)   # [d, m]
        wsh1 = np.zeros((d, shw), np.float32)
        wsh3 = np.zeros((d, shw), np.float32)
        t0 = OWN_START[c] * 128
        nh = OWN_N[c] * 128
        wsh1[:, :nh] = w1t[:, t0:t0 + nh]
        wsh3[:, :nh] = w3t[:, t0:t0 + nh]
        in_maps.append({
            "xT": xc,
            "wsh1": wsh1,
            "wsh3": wsh3,
            "w0r1": s0_1,
            "w0r3": s0_3,
            "wsh2": np.ascontiguousarray(
                w2t[c * r2:(c + 1) * r2].reshape(128, sw)),
        })
    return in_maps


def kernel(x, w1, w3, w2):
    from concourse.bass_utils import run_bass_kernel_spmd

    nc = _get_module()
    in_maps = prep_inputs(x, w1, w3, w2)
    res = run_bass_kernel_spmd(nc, in_maps, core_ids=list(range(N_CORES)))
    _NC_CACHE["last_results"] = res
    yf = np.concatenate([r["y"] for r in res.results], axis=0)  # [16384, 2048]
    return np.ascontiguousarray(yf.reshape(B, S, D).astype(np.float32))
